# revision 9
# baseline (speedup 1.0000x reference)
"""Causal single-head attention (B=8, S=2048, D=1024, fp32) on 8 NeuronCores.

Data-parallel over batch: one batch element per core, weights replicated.
Per-core pipeline (all matmuls in fp32r — full-rate on the PE):
  1. xT = x.T via PE transposes                       [din, s]
  2. QT = Wq.T @ xT, KT = Wk.T @ xT  (spilled to DRAM scratch)
     V  = x @ Wv                      (spilled to DRAM scratch)
  3. per 512-wide query chunk c:
       S^T[k-tile, q-chunk] accumulated over 8 e-tiles of KT/QT
       causal mask added in PSUM via identity @ mask_const (-1e9)
       P^T = exp(S^T / 32) on ScalarE (masked entries underflow to 0)
       rowsum_q = P^T.T @ ones   (N=2 matmuls)
       O[q, :] = (P^T.T @ V) * (1/rowsum)
"""

import numpy as np

B, S, D = 8, 2048, 1024
P = 128
NCORES = 8

_built = None


def _build():
    import concourse.tile as tile
    import concourse.mybir as mybir
    from concourse import bacc

    FP32 = mybir.dt.float32
    FP32R = mybir.dt.float32r
    AF = mybir.ActivationFunctionType

    nc = bacc.Bacc("TRN2", target_bir_lowering=False, debug=False, num_devices=NCORES)
    x_d = nc.dram_tensor("x", [S, D], FP32R, kind="ExternalInput").ap()
    wq_d = nc.dram_tensor("Wq", [D, D], FP32R, kind="ExternalInput").ap()
    wk_d = nc.dram_tensor("Wk", [D, D], FP32R, kind="ExternalInput").ap()
    wv_d = nc.dram_tensor("Wv", [D, D], FP32R, kind="ExternalInput").ap()
    out_d = nc.dram_tensor("out", [S, D], FP32, kind="ExternalOutput").ap()
    # DRAM scratch
    qt_d = nc.dram_tensor("qt_s", [8, P, S], FP32R, kind="Internal").ap()
    kt_d = nc.dram_tensor("kt_s", [16, P, 8, P], FP32R, kind="Internal").ap()
    v_d = nc.dram_tensor("v_s", [16, P, D], FP32R, kind="Internal").ap()

    ident_c = nc.inline_tensor(np.eye(P, dtype=np.float32), name="ident_c")
    ones_c = nc.inline_tensor(np.ones((P, 2), dtype=np.float32), name="ones_c")
    # sliding causal mask: tile j uses cols [(3-j)*128, (3-j)*128+512);
    # value at (p, y) = 0 iff y >= p + 128j else -1e9
    zz = np.arange(896)[None, :]
    pp = np.arange(P)[:, None]
    masks_np = np.where(zz >= pp + 384, 0.0, -1e9).astype(np.float32)
    masks_c = nc.inline_tensor(masks_np, name="masks_c")

    with tile.TileContext(nc) as tc:
        with (
            tc.tile_pool(name="big", bufs=1) as big,      # xT then V (64KB/part)
            tc.tile_pool(name="wst", bufs=16) as wst,     # W slices (B) / est (C)
            tc.tile_pool(name="qtc", bufs=2) as qtp,      # 2x16KB
            tc.tile_pool(name="osb", bufs=2) as osb,      # 2x4KB
            tc.tile_pool(name="xin", bufs=7) as xin,      # x tiles (A) / kt slices (C)
            tc.tile_pool(name="stage", bufs=2) as stage,  # 2x2KB
            tc.tile_pool(name="smalls", bufs=1) as smalls,
            tc.tile_pool(name="rcp", bufs=2) as rcp,
            tc.tile_pool(name="ps", bufs=6, space="PSUM") as ps,
            tc.tile_pool(name="ps1", bufs=2, space="PSUM") as ps1,
        ):
            ident = smalls.tile([P, P], FP32R, tag="ident")
            nc.sync.dma_start(out=ident, in_=ident_c.ap().bitcast(FP32R))
            ones = smalls.tile([P, 2], FP32R, tag="ones")
            nc.sync.dma_start(out=ones, in_=ones_c.ap().bitcast(FP32R))
            mask_sb = smalls.tile([P, 896], FP32R, tag="masks")
            nc.scalar.dma_start(out=mask_sb, in_=masks_c.ap().bitcast(FP32R))

            # ---- Phase A+B fused: per s-chunk transposes + Q projection ----
            xt = big.tile([P, 8, S], FP32R, tag="big")

            def load_x(si):
                x_tile = xin.tile([P, D], FP32R, tag="xin")
                eng = (nc.sync, nc.scalar)[si % 2]
                eng.dma_start(out=x_tile, in_=x_d[si * P:(si + 1) * P, :])
                return x_tile

            x_pend = {si: load_x(si) for si in range(4)}  # chunk 0 first
            wq_sb = []
            for kd in range(8):
                w_t = wst.tile([P, D], FP32R, tag="wst")
                (nc.sync, nc.scalar)[kd % 2].dma_start(
                    out=w_t, in_=wq_d[kd * P:(kd + 1) * P, :]
                )
                wq_sb.append(w_t)

            for sc in range(4):
                for si in range(4 * sc, 4 * sc + 4):
                    x_tile = x_pend.pop(si)
                    for kd in range(8):
                        tp = ps.tile([P, P], FP32R, tag="ps")
                        nc.tensor.transpose(tp, x_tile[:, kd * P:(kd + 1) * P], ident)
                        nc.vector.tensor_copy(
                            out=xt[:, kd, si * P:(si + 1) * P], in_=tp
                        )
                if sc < 3:
                    for si in range(4 * sc + 4, 4 * sc + 8):
                        x_pend[si] = load_x(si)
                # Q projection for this s-chunk
                for et in range(8):
                    pst = ps.tile([P, 512], FP32, tag="ps")
                    for kd in range(8):
                        nc.tensor.matmul(
                            pst,
                            lhsT=wq_sb[kd][:, et * P:(et + 1) * P],
                            rhs=xt[:, kd, sc * 512:(sc + 1) * 512],
                            start=(kd == 0),
                            stop=(kd == 7),
                        )
                    st = stage.tile([P, 512], FP32R, tag="stage")
                    nc.vector.tensor_copy(out=st, in_=pst)
                    nc.gpsimd.dma_start(
                        out=qt_d[et, :, sc * 512:(sc + 1) * 512], in_=st
                    )

            # ---- K and V projections ----
            for w_d, which in ((wk_d, "k"), (wv_d, "v")):
                w_sb = []
                for kd in range(8):
                    w_t = wst.tile([P, D], FP32R, tag="wst")
                    (nc.sync, nc.scalar)[kd % 2].dma_start(
                        out=w_t, in_=w_d[kd * P:(kd + 1) * P, :]
                    )
                    w_sb.append(w_t)
                if which == "k":
                    for et in range(8):
                        for sc in range(4):
                            pst = ps.tile([P, 512], FP32, tag="ps")
                            for kd in range(8):
                                nc.tensor.matmul(
                                    pst,
                                    lhsT=w_sb[kd][:, et * P:(et + 1) * P],
                                    rhs=xt[:, kd, sc * 512:(sc + 1) * 512],
                                    start=(kd == 0),
                                    stop=(kd == 7),
                                )
                            st = stage.tile([P, 512], FP32R, tag="stage")
                            nc.vector.tensor_copy(out=st, in_=pst)
                            nc.gpsimd.dma_start(
                                out=kt_d[4 * sc:4 * sc + 4, :, et, :].rearrange(
                                    "ks ki sl -> ki ks sl"
                                ),
                                in_=st,
                            )
                else:
                    for st_i in range(16):
                        for ec in range(2):
                            pst = ps.tile([P, 512], FP32, tag="ps")
                            for kd in range(8):
                                nc.tensor.matmul(
                                    pst,
                                    lhsT=xt[:, kd, st_i * P:(st_i + 1) * P],
                                    rhs=w_sb[kd][:, ec * 512:(ec + 1) * 512],
                                    start=(kd == 0),
                                    stop=(kd == 7),
                                )
                            st = stage.tile([P, 512], FP32R, tag="stage")
                            nc.vector.tensor_copy(out=st, in_=pst)
                            nc.gpsimd.dma_start(
                                out=v_d[st_i, :, ec * 512:(ec + 1) * 512], in_=st
                            )

            # ---- Phase C: attention ----
            v_sb = big.tile([P, 16, D], FP32R, tag="big")

            for c in range(4):
                for g in (2 * c, 2 * c + 1):
                    nc.gpsimd.dma_start(
                        out=v_sb[:, 2 * g:2 * g + 2, :],
                        in_=v_d[2 * g:2 * g + 2].rearrange("k ki e -> ki k e"),
                    )
                qt_sb = qtp.tile([P, 8, 512], FP32R, tag="qtc")
                for g in range(2):
                    nc.scalar.dma_start(
                        out=qt_sb[:, 4 * g:4 * g + 4, :],
                        in_=qt_d[4 * g:4 * g + 4, :, c * 512:(c + 1) * 512].rearrange(
                            "e ki s -> ki e s"
                        ),
                    )
                nk = 4 * c + 4
                est_tiles = {}
                for k in range(nk):
                    kt_sl = xin.tile([P, 8, P], FP32R, tag="xin")
                    (nc.sync, nc.scalar)[k % 2].dma_start(out=kt_sl, in_=kt_d[k])
                    j = k - 4 * c
                    sps = ps.tile([P, 512], FP32, tag="ps")
                    for e in range(8):
                        nc.tensor.matmul(
                            sps,
                            lhsT=kt_sl[:, e, :],
                            rhs=qt_sb[:, e, :],
                            start=(e == 0),
                            stop=(e == 7 and j < 0),
                        )
                    if j >= 0:
                        # diagonal block: add -1e9 outside the causal region
                        nc.tensor.matmul(
                            sps, lhsT=ident, rhs=mask_sb[:, (3 - j) * P:(3 - j) * P + 512],
                            start=False, stop=True,
                        )
                    est = wst.tile([P, 512], FP32R, tag="wst")
                    nc.scalar.activation(out=est, in_=sps, func=AF.Exp, scale=0.03125)
                    est_tiles[k] = est
                for j in range(4):
                    q_abs = 4 * c + j
                    rs = ps1.tile([P, 2], FP32, tag="ps1")
                    for k in range(q_abs + 1):
                        nc.tensor.matmul(
                            rs,
                            lhsT=est_tiles[k][:, j * P:(j + 1) * P],
                            rhs=ones,
                            start=(k == 0),
                            stop=(k == q_abs),
                        )
                    rec = rcp.tile([P, 1], FP32, tag="rcp")
                    nc.vector.reciprocal(rec, rs[:, 0:1])
                    o_sb = osb.tile([P, D], FP32, tag="osb")
                    for h in range(2):
                        ops_t = ps.tile([P, 512], FP32, tag="ps")
                        for k in range(q_abs + 1):
                            nc.tensor.matmul(
                                ops_t,
                                lhsT=est_tiles[k][:, j * P:(j + 1) * P],
                                rhs=v_sb[:, k, h * 512:(h + 1) * 512],
                                start=(k == 0),
                                stop=(k == q_abs),
                            )
                        nc.vector.tensor_scalar_mul(
                            o_sb[:, h * 512:(h + 1) * 512], ops_t, rec
                        )
                    nc.gpsimd.dma_start(
                        out=out_d[q_abs * P:(q_abs + 1) * P, :], in_=o_sb
                    )

    nc.compile()
    return nc


def _get_nc():
    global _built
    if _built is None:
        _built = _build()
    return _built


def _run(inputs, trace=False):
    from concourse.bass_utils import run_bass_kernel_spmd

    x = inputs["x"]
    in_maps = [
        {
            "x": np.ascontiguousarray(x[c], dtype=np.float32),
            "Wq": np.asarray(inputs["Wq"], dtype=np.float32),
            "Wk": np.asarray(inputs["Wk"], dtype=np.float32),
            "Wv": np.asarray(inputs["Wv"], dtype=np.float32),
        }
        for c in range(NCORES)
    ]
    res = run_bass_kernel_spmd(
        nc=_get_nc(), in_maps=in_maps, core_ids=list(range(NCORES)), trace=trace
    )
    out = np.stack([res.results[c]["out"] for c in range(NCORES)], axis=0)
    return out, res


def kernel(x, Wq, Wk, Wv):
    out, _ = _run({"x": x, "Wq": Wq, "Wk": Wk, "Wv": Wv}, trace=False)
    return out
